# revision 1
# baseline (speedup 1.0000x reference)
"""GQA attention (RoPE, no mask) sharded over 8 NeuronCores.

Sharding: TP over the 4 KV-head groups x DP over batch (2).
core c -> batch b = c//4, kv-group g = c%4 (query heads 4g..4g+3).
Each core computes Q/K/V projections for its heads, RoPE, softmax(QK^T)V,
and its o_proj partial; the 4 partials per batch are summed host-side.

All matmuls run in bf16 (f32 PSUM accumulation); softmax runs in f32.
Scores are computed K-major (ST = [s, q]) so the exp output feeds the PV
matmul directly; softmax denominators use a DVE accumulator + ones-matmul
partition reduce + ones-matmul broadcast of the reciprocal.
"""

import sys

sys.path.insert(0, "/opt/trn_rl_repo")

from contextlib import ExitStack

import ml_dtypes
import numpy as np

import concourse.bass as bass
import concourse.tile as tile
from concourse import bacc, mybir
from concourse.bass_utils import run_bass_kernel_spmd

BF16 = mybir.dt.bfloat16
F32 = mybir.dt.float32
NP_BF16 = ml_dtypes.bfloat16

B, T_FULL, S_FULL, D_FULL = 2, 2048, 2048, 2048
N_HEADS, KV_HEADS, H = 16, 4, 128
HG = N_HEADS // KV_HEADS  # query heads per core (4)
HD = HG * H  # per-core q head dims (512)
MIN_TS, MAX_TS = 1.0, 10000.0


def build(T=T_FULL, S=S_FULL, D=D_FULL, repeat=1):
    """Build the per-core Bass graph. Returns compiled nc."""
    assert T % 512 == 0 and S % 512 == 0 and D % 128 == 0
    TQC = T // 512  # q chunks of 512
    SC = S // 512  # s chunks of 512
    S128 = S // 128  # s chunks of 128
    DC = D // 128  # contraction chunks of 128

    nc = bacc.Bacc("TRN2", target_bir_lowering=False, debug=False, num_devices=8)

    # All inputs are host-prelayouted so every DMA is contiguous per partition.
    id_d = nc.dram_tensor("ident", [128, 128], BF16, kind="ExternalInput").ap()
    xq_d = nc.dram_tensor("XqT", [T // 512, 128, DC, 512], BF16, kind="ExternalInput").ap()
    xkv_d = nc.dram_tensor("XkvT", [S // 512, 128, DC, 512], BF16, kind="ExternalInput").ap()
    wq_d = nc.dram_tensor("Wq", [128, DC, HD], BF16, kind="ExternalInput").ap()
    wk_d = nc.dram_tensor("Wk", [128, DC, H], BF16, kind="ExternalInput").ap()
    wv_d = nc.dram_tensor("Wv", [128, DC, H], BF16, kind="ExternalInput").ap()
    wo_d = nc.dram_tensor("Wo", [128, HG, D], BF16, kind="ExternalInput").ap()
    cosq_d = nc.dram_tensor("cos_q", [H // 2, T], F32, kind="ExternalInput").ap()
    sinq_d = nc.dram_tensor("sin_q", [H // 2, T], F32, kind="ExternalInput").ap()
    cosk_d = nc.dram_tensor("cos_k", [H // 2, S], F32, kind="ExternalInput").ap()
    sink_d = nc.dram_tensor("sin_k", [H // 2, S], F32, kind="ExternalInput").ap()
    out_d = nc.dram_tensor("out", [T, D], F32, kind="ExternalOutput").ap()

    with tile.TileContext(nc) as tc, ExitStack() as ctx:
        wpool = ctx.enter_context(tc.tile_pool(name="w", bufs=1))
        xpool = ctx.enter_context(tc.tile_pool(name="x", bufs=3))
        qkv = ctx.enter_context(tc.tile_pool(name="qkv", bufs=1))
        ptp = ctx.enter_context(tc.tile_pool(name="pt", bufs=8))
        accp = ctx.enter_context(tc.tile_pool(name="acc", bufs=2))
        tmpp = ctx.enter_context(tc.tile_pool(name="tmp", bufs=4))
        outp = ctx.enter_context(tc.tile_pool(name="outs", bufs=2))
        ps_st = ctx.enter_context(tc.tile_pool(name="ps_st", bufs=4, space="PSUM"))
        ps_ot = ctx.enter_context(tc.tile_pool(name="ps_ot", bufs=2, space="PSUM"))
        ps_sm = ctx.enter_context(tc.tile_pool(name="ps_sm", bufs=2, space="PSUM"))

        # ---- weights / tables (K/V-projection path loads first) ----
        wk_sb = wpool.tile([128, DC, H], BF16, tag="wk")
        nc.sync.dma_start(wk_sb[:], wk_d[:])
        wv_sb = wpool.tile([128, DC, H], BF16, tag="wv")
        nc.sync.dma_start(wv_sb[:], wv_d[:])
        # cos/sin packed: rows 0:64 = q tables, 64:128 = k tables
        cos_sb = wpool.tile([128, max(T, S)], F32, tag="cos")
        sin_sb = wpool.tile([128, max(T, S)], F32, tag="sin")
        nc.sync.dma_start(cos_sb[64:128, 0:S], cosk_d[:])
        nc.sync.dma_start(sin_sb[64:128, 0:S], sink_d[:])
        ident = wpool.tile([128, 128], BF16, tag="ident")
        nc.sync.dma_start(ident[:], id_d[:])
        wq_sb = wpool.tile([128, DC, HD], BF16, tag="wq")
        nc.scalar.dma_start(wq_sb[:], wq_d[:])
        nc.scalar.dma_start(cos_sb[0:64, 0:T], cosq_d[:])
        nc.gpsimd.dma_start(sin_sb[0:64, 0:T], sinq_d[:])
        wo_sb = wpool.tile([128, HG, D], BF16, tag="wo")
        nc.gpsimd.dma_start(wo_sb[:], wo_d[:])
        ones_col = wpool.tile([128, 1], BF16, tag="ones_col")  # lhsT for partition sum
        nc.vector.memset(ones_col[:], 1.0)
        ones_f32 = wpool.tile([128, 1], F32, tag="ones_f32")  # lhsT for f32 acc reduce
        nc.vector.memset(ones_f32[:], 1.0)

        qt_sb = qkv.tile([128, HG, T], BF16, tag="qt")
        kt_sb = qkv.tile([128, S], BF16, tag="kt")
        vt_sb = qkv.tile([128, S], BF16, tag="vt")
        v_sb = qkv.tile([128, S128, H], BF16, tag="v")
        ot_sb = qkv.tile([128, HG, T], BF16, tag="ot")

        def rope(dst, ps, cos_ap, sin_ap):
            # dst[0:64] = ps[0:64]*cos - ps[64:128]*sin
            # dst[64:128] = ps[64:128]*cos + ps[0:64]*sin
            n = ps.shape[-1]
            t1 = tmpp.tile([64, 512], F32, tag="t1")
            t2 = tmpp.tile([64, 512], F32, tag="t2")
            nc.vector.tensor_mul(t1[:, 0:n], ps[0:64, :], cos_ap)
            nc.vector.tensor_mul(t2[:, 0:n], ps[64:128, :], sin_ap)
            nc.vector.tensor_sub(dst[0:64, :], t1[:, 0:n], t2[:, 0:n])
            t3 = tmpp.tile([64, 512], F32, tag="t1")
            t4 = tmpp.tile([64, 512], F32, tag="t2")
            nc.vector.tensor_mul(t3[:, 0:n], ps[64:128, :], cos_ap)
            nc.vector.tensor_mul(t4[:, 0:n], ps[0:64, :], sin_ap)
            nc.vector.tensor_add(dst[64:128, :], t3[:, 0:n], t4[:, 0:n])

        def body():
            def emit_transposes(j):
                # V[s, h] = transpose of VT[h, s] per 128x128 block
                for sub in range(4):
                    pst_tr = ps_st.tile([128, H], BF16, tag="st")
                    nc.tensor.transpose(
                        pst_tr[:], vt_sb[:, bass.ts(4 * j + sub, 128)], ident[:]
                    )
                    nc.scalar.copy(v_sb[:, 4 * j + sub, :], pst_tr[:])

            # ---- K/V projections ----
            for j in range(SC):
                xk = xpool.tile([128, DC, 512], BF16, tag="x")
                nc.sync.dma_start(xk[:], xkv_d[j])
                psk = ps_st.tile([128, 512], F32, tag="st")
                for d in range(DC):
                    nc.tensor.matmul(
                        psk[:], wk_sb[:, d, :], xk[:, d, :],
                        start=(d == 0), stop=(d == DC - 1),
                    )
                rope(
                    kt_sb[:, bass.ts(j, 512)], psk,
                    cos_sb[64:128, bass.ts(j, 512)], sin_sb[64:128, bass.ts(j, 512)],
                )
                psv = ps_st.tile([128, 512], F32, tag="st")
                for d in range(DC):
                    nc.tensor.matmul(
                        psv[:], wv_sb[:, d, :], xk[:, d, :],
                        start=(d == 0), stop=(d == DC - 1),
                    )
                nc.scalar.copy(vt_sb[:, bass.ts(j, 512)], psv[:])
                if j > 0:
                    emit_transposes(j - 1)
            emit_transposes(SC - 1)

            # ---- per q-chunk: attention + o_proj; Q proj hoisted one
            # chunk ahead so its matmuls cover the softmax scale-chain
            # latency that o_proj's last heads wait on ----
            def qproj(qc):
                xq = xpool.tile([128, DC, 512], BF16, tag="x")
                nc.sync.dma_start(xq[:], xq_d[qc])
                for hh in range(HG):
                    psq = ps_st.tile([128, 512], F32, tag="st")
                    for d in range(DC):
                        nc.tensor.matmul(
                            psq[:], wq_sb[:, d, bass.ts(hh, 128)], xq[:, d, :],
                            start=(d == 0), stop=(d == DC - 1),
                        )
                    rope(
                        qt_sb[:, hh, bass.ts(qc, 512)], psq,
                        cos_sb[0:64, bass.ts(qc, 512)], sin_sb[0:64, bass.ts(qc, 512)],
                    )

            qproj(0)
            for qc in range(TQC):
                for hp in range(HG // 2):  # head pairs share KT_s/ones/V_s lhsT
                    h0, h1 = 2 * hp, 2 * hp + 1
                    pso0 = ps_ot.tile([128, 512], F32, tag="ot")
                    pso1 = ps_ot.tile([128, 512], F32, tag="ot")
                    psum0 = ps_sm.tile([1, 512], F32, tag="sm")
                    acc1 = accp.tile([128, 512], F32, tag="acc1")
                    # Software-pipelined: emit ST for s+1 before the consumers
                    # of exp_s so PE never stalls on ACT (skew-1).
                    st_tiles = [None] * S128

                    def emit_st(s):
                        pst0 = ps_st.tile([128, 512], F32, tag="st", name=f"pst0_{s}")
                        pst1 = ps_st.tile([128, 512], F32, tag="st", name=f"pst1_{s}")
                        nc.tensor.matmul(
                            pst0[:], kt_sb[:, bass.ts(s, 128)],
                            qt_sb[:, h0, bass.ts(qc, 512)], start=True, stop=True,
                        )
                        nc.tensor.matmul(
                            pst1[:], kt_sb[:, bass.ts(s, 128)],
                            qt_sb[:, h1, bass.ts(qc, 512)], start=True, stop=True,
                        )
                        st_tiles[s] = (pst0, pst1)

                    emit_st(0)
                    for s in range(S128):
                        if s + 1 < S128:
                            emit_st(s + 1)
                        pst0, pst1 = st_tiles[s]
                        st_tiles[s] = None
                        pt0 = ptp.tile([128, 512], BF16, tag="pt")
                        pt1 = ptp.tile([128, 512], BF16, tag="pt")
                        nc.scalar.activation(
                            pt0[:], pst0[:], mybir.ActivationFunctionType.Exp
                        )
                        nc.scalar.activation(
                            pt1[:], pst1[:], mybir.ActivationFunctionType.Exp
                        )
                        # softmax sums: head0 on PE (ones matmul), head1 on DVE
                        nc.tensor.matmul(
                            psum0[:], ones_col[:], pt0[:],
                            start=(s == 0), stop=(s == S128 - 1),
                        )
                        if s == 0:
                            nc.vector.tensor_copy(acc1[:], pt1[:])
                        else:
                            nc.vector.tensor_add(acc1[:], acc1[:], pt1[:])
                        nc.tensor.matmul(
                            pso0[:], v_sb[:, s, :], pt0[:],
                            start=(s == 0), stop=(s == S128 - 1),
                        )
                        nc.tensor.matmul(
                            pso1[:], v_sb[:, s, :], pt1[:],
                            start=(s == 0), stop=(s == S128 - 1),
                        )
                    psum1 = ps_sm.tile([1, 512], F32, tag="sm")
                    nc.tensor.matmul(psum1[:], ones_f32[:], acc1[:], start=True, stop=True)
                    # reciprocal + gpsimd partition-broadcast, then scale
                    for hh, pso, pssum in ((h0, pso0, psum0), (h1, pso1, psum1)):
                        rec = accp.tile([1, 512], F32, tag="rec")
                        nc.vector.reciprocal(rec[:], pssum[:])
                        rbc = accp.tile([128, 512], F32, tag="rbc")
                        nc.gpsimd.partition_broadcast(rbc[:], rec[:])
                        nc.vector.tensor_mul(
                            ot_sb[:, hh, bass.ts(qc, 512)], pso[:], rbc[:]
                        )

                if qc + 1 < TQC:
                    qproj(qc + 1)

                # ---- o_proj for this q chunk ----
                for tsub in range(4):
                    trow = qc * 512 + tsub * 128
                    ostage = outp.tile([128, D], F32, tag="ostage")
                    for dc2 in range(D // 512):
                        pso2 = ps_st.tile([128, 512], F32, tag="st")
                        for hh in range(HG):
                            nc.tensor.matmul(
                                pso2[:],
                                ot_sb[:, hh, trow : trow + 128],
                                wo_sb[:, hh, bass.ts(dc2, 512)],
                                start=(hh == 0), stop=(hh == HG - 1),
                            )
                        nc.vector.tensor_copy(ostage[:, bass.ts(dc2, 512)], pso2[:])
                    nc.sync.dma_start(out_d[trow : trow + 128, :], ostage[:])

        if repeat == 1:
            body()
        else:
            with tc.For_i(0, repeat):
                body()

    nc.compile()
    return nc


def _shard_inputs(Xq, Xkv, q_positions, kv_positions, Wq, Wk, Wv, Wo):
    """Build per-core input maps. Core c: batch c//4, kv-group c%4."""
    D = Xq.shape[2]
    half = H // 2
    frac = 2.0 * np.arange(half, dtype=np.float32) / H
    ts = (MIN_TS * (MAX_TS / MIN_TS) ** frac).astype(np.float32)

    def tables(pos):
        s = pos.astype(np.float32)[None, :] / ts[:, None]
        return np.cos(s).astype(np.float32), np.sin(s).astype(np.float32)

    DC = D // 128

    def chunked_xT(X):
        # [L, D] -> X.T laid out as [L//512, 128, DC, 512]: contiguous per partition
        xt = np.ascontiguousarray(X.T).astype(NP_BF16)  # [D, L]
        L = X.shape[0]
        return np.ascontiguousarray(
            xt.reshape(DC, 128, L // 512, 512).transpose(2, 1, 0, 3)
        )

    def chunked_w(W, m):
        # [D, m] -> [128, DC, m]
        return np.ascontiguousarray(
            W.reshape(DC, 128, m).transpose(1, 0, 2)
        ).astype(NP_BF16)

    in_maps = []
    for c in range(8):
        b, g = c // 4, c % 4
        cq, sq = tables(q_positions[b])
        ck, sk = tables(kv_positions[b])
        in_maps.append(
            {
                "XqT": chunked_xT(Xq[b]),
                "XkvT": chunked_xT(Xkv[b]),
                "Wq": chunked_w(Wq[:, HG * g : HG * (g + 1), :].reshape(D, HD), HD),
                "Wk": chunked_w(Wk[:, g, :], H),
                "Wv": chunked_w(Wv[:, g, :], H),
                # Wo [HG, 128, D] -> [128, HG, D]: wo_sb[h, hh, d] = Wo[g*HG+hh, h, d]
                "Wo": np.ascontiguousarray(
                    Wo[HG * g : HG * (g + 1)].transpose(1, 0, 2)
                ).astype(NP_BF16),
                "cos_q": cq, "sin_q": sq, "cos_k": ck, "sin_k": sk,
                "ident": np.eye(128, dtype=NP_BF16),
            }
        )
    return in_maps


_NC_CACHE = {}


def kernel(Xq, Xkv, q_positions, kv_positions, Wq, Wk, Wv, Wo):
    key = ("full", 1)
    if key not in _NC_CACHE:
        _NC_CACHE[key] = build()
    nc = _NC_CACHE[key]
    in_maps = _shard_inputs(Xq, Xkv, q_positions, kv_positions, Wq, Wk, Wv, Wo)
    res = run_bass_kernel_spmd(nc, in_maps, core_ids=list(range(8)))
    T, D = Xq.shape[1], Xq.shape[2]
    out = np.zeros((B, T, D), dtype=np.float32)
    for c in range(8):
        out[c // 4] += res.results[c]["out"]
    return out

